# revision 22
# baseline (speedup 1.0000x reference)
"""CenterLoss update kernel for Trainium2, 8-core SPMD — class-sharded (baseline control)."""

import numpy as np

N, C, D = 16384, 10000, 128
NCORES = 8
CS = C // NCORES   # 1250 classes per core
LR = 0.5
P = 128
KT = N // P        # 128 virtual k-tiles
NPAIR = KT // 2    # 64 pair-tiles
NT3 = (CS + P - 1) // P  # output tiles over the class shard
PCHUNKS = [1024] * 16   # preds load chunks (cols of the [128, N] natural view)
QQSPLIT = 56            # counts accumulator split point (pair index)
assert sum(PCHUNKS) == KT * D


def _chunks(width, step=512):
    out = []
    c0 = 0
    while c0 < width:
        out.append((c0, min(step, width - c0)))
        c0 += step
    return out


def build_program(cs=CS, d=D, kt=KT):
    """Build the SPMD Bass program (identical on every core)."""
    import concourse.bacc as bacc
    import concourse.mybir as mybir
    import concourse.tile as tile
    from concourse.masks import make_identity

    f32 = mybir.dt.float32
    f32r = mybir.dt.float32r
    mult = mybir.AluOpType.mult
    add = mybir.AluOpType.add

    n = kt * P
    nt3 = NT3
    npair = NPAIR
    assert cs * 4 <= 3 * 2048, "S.T PSUM tile must fit in 3 banks"

    nc = bacc.Bacc(
        "TRN2",
        target_bir_lowering=False,
        debug=False,
        num_devices=NCORES,
    )

    preds = nc.dram_tensor("preds", [P, kt * d], f32r, kind="ExternalInput").ap()
    labels2 = nc.dram_tensor(
        "labels", [n // 2, 2 * cs], f32r, kind="ExternalInput"
    ).ap()
    center = nc.dram_tensor("center", [P, nt3 * d], f32, kind="ExternalInput").ap()
    out = nc.dram_tensor("out", [cs, d], f32, kind="ExternalOutput").ap()

    trigger_qq = {}
    for cch in range(len(PCHUNKS)):
        trigger_qq.setdefault(max(0, 4 * cch - 2), []).append(cch)

    with tile.TileContext(nc) as tc:
        with tc.tile_pool(name="const", bufs=1) as const_pool:
            identity = const_pool.tile([P, P], f32, name="identity")
            make_identity(nc, identity[:])
            ones_col = const_pool.tile([P, 1], f32, name="ones_col")
            nc.vector.memset(ones_col[:], 1.0)

            ctr_sb = const_pool.tile([P, nt3 * d], f32, name="ctr_sb")
            nc.gpsimd.dma_start(out=ctr_sb[:], in_=center[:])

            preds_hi = [
                const_pool.tile([P, pw], f32r, name=f"preds_hi_{cch}")
                for cch, pw in enumerate(PCHUNKS)
            ]
            pstart = [sum(PCHUNKS[:cch]) for cch in range(len(PCHUNKS))]

            acc_a = const_pool.tile([P, 2 * cs], f32, name="acc_a")
            acc_c = const_pool.tile([P, 2 * cs], f32, name="acc_c")

            st_sb = const_pool.tile([d, cs], f32, name="st_sb")
            cnt_row = const_pool.tile([1, cs], f32, name="cnt_row")

            with (
                tc.tile_pool(name="lab", bufs=7) as lab_pool,
                tc.tile_pool(name="psum1", bufs=1, space="PSUM") as psum1,
            ):
                st_psum = psum1.tile([d, cs], f32, name="st_psum", space="PSUM")
                cnt_psum = psum1.tile([1, cs], f32, name="cnt_psum", space="PSUM")
                for qq in range(npair):
                    for cch in trigger_qq.get(qq, []):
                        peng = nc.sync if cch % 2 == 0 else nc.scalar
                        peng.dma_start(
                            out=preds_hi[cch][:],
                            in_=preds[:, pstart[cch]:pstart[cch] + PCHUNKS[cch]],
                        )
                    lab2 = lab_pool.tile(
                        [P, 2 * cs], f32r, name=f"lab_{qq}", tag="lab"
                    )
                    eng = nc.sync if qq % 2 == 0 else nc.scalar
                    eng.dma_start(out=lab2[:], in_=labels2[qq::npair, :])
                    if qq == npair - 1:
                        for h in (0, 1):
                            for c0, w in _chunks(cs):
                                nc.tensor.matmul(
                                    out=cnt_psum[0:1, c0:c0 + w],
                                    lhsT=ones_col[:],
                                    rhs=acc_c[:, h * cs + c0:h * cs + c0 + w],
                                    start=False,
                                    stop=False,
                                )
                    for h in (0, 1):
                        q = 2 * qq + h
                        col = q * d
                        cch = max(
                            i for i in range(len(PCHUNKS)) if pstart[i] <= col
                        )
                        for c0, w in _chunks(cs):
                            nc.tensor.matmul(
                                out=st_psum[:, c0:c0 + w],
                                lhsT=preds_hi[cch][:, col - pstart[cch]:
                                                   col - pstart[cch] + d],
                                rhs=lab2[:, h * cs + c0:h * cs + c0 + w],
                                start=(q == 0),
                                stop=(q == kt - 1),
                            )
                    if qq < npair - 1:
                        acc = acc_a if qq < QQSPLIT else acc_c
                        if qq in (0, QQSPLIT):
                            nc.vector.tensor_copy(
                                out=acc[:], in_=lab2[:].bitcast(f32)
                            )
                        else:
                            nc.vector.tensor_add(
                                out=acc[:], in0=acc[:], in1=lab2[:].bitcast(f32)
                            )
                    else:
                        lab_last = lab2
                    if qq == QQSPLIT + 2:
                        for h in (0, 1):
                            for c0, w in _chunks(cs):
                                nc.tensor.matmul(
                                    out=cnt_psum[0:1, c0:c0 + w],
                                    lhsT=ones_col[:],
                                    rhs=acc_a[:, h * cs + c0:h * cs + c0 + w],
                                    start=(h == 0),
                                    stop=False,
                                )

                for c0, w in _chunks(cs):
                    for h in (0, 1):
                        nc.tensor.matmul(
                            out=cnt_psum[0:1, c0:c0 + w],
                            lhsT=ones_col[:],
                            rhs=lab_last[:, h * cs + c0:h * cs + c0 + w]
                                .bitcast(f32),
                            start=False,
                            stop=(h == 1),
                        )
                    nc.scalar.copy(
                        out=cnt_row[0:1, c0:c0 + w],
                        in_=cnt_psum[0:1, c0:c0 + w],
                    )
                nc.vector.tensor_copy(out=st_sb[:], in_=st_psum[:])

            with (
                tc.tile_pool(name="p3", bufs=2) as p3,
                tc.tile_pool(name="psum3", bufs=1, space="PSUM") as psum3,
            ):
                cnt_all = psum3.tile([P, nt3], f32, name="cnt_all", space="PSUM")
                for tt in range(nt3):
                    w = min(P, cs - tt * P)
                    nc.tensor.transpose(
                        out=cnt_all[0:w, tt:tt + 1],
                        in_=cnt_row[0:1, tt * P:tt * P + w],
                        identity=identity[0:1, 0:1],
                    )
                den = p3.tile([P, nt3], f32, name="den", tag="den", bufs=1)
                nc.vector.tensor_scalar_add(out=den[:], in0=cnt_all[:], scalar1=1.0)
                rec = p3.tile([P, nt3], f32, name="rec", tag="rec", bufs=1)
                nc.vector.reciprocal(out=rec[:], in_=den[:])
                gam = p3.tile([P, nt3], f32, name="gam", tag="gam", bufs=1)
                nc.vector.tensor_scalar_mul(out=gam[:], in0=rec[:], scalar1=0.5)
                bet = p3.tile([P, nt3], f32, name="bet", tag="bet", bufs=1)
                nc.vector.tensor_tensor(
                    out=bet[:], in0=cnt_all[:], in1=rec[:], op=mult
                )
                nc.vector.tensor_scalar(
                    out=bet[:], in0=bet[:],
                    scalar1=-0.5, scalar2=1.0, op0=mult, op1=add,
                )

                o1_all = p3.tile([P, nt3 * d], f32, name="o1_all", tag="o1",
                                 bufs=1)
                ou_all = p3.tile([P, nt3 * d], f32, name="ou_all", tag="ou",
                                 bufs=1)
                for tt in range(nt3):
                    w = min(P, cs - tt * P)
                    nc.scalar.mul(
                        out=o1_all[0:w, tt * d:tt * d + d],
                        in_=ctr_sb[0:w, tt * d:tt * d + d],
                        mul=bet[0:w, tt:tt + 1],
                    )
                    trp = psum3.tile([P, d], f32, name=f"trp_{tt}", tag="trp",
                                     bufs=4, space="PSUM")
                    nc.tensor.transpose(
                        out=trp[0:w, 0:d],
                        in_=st_sb[:, tt * P:tt * P + w],
                        identity=identity[:, 0:d],
                    )
                    nc.vector.scalar_tensor_tensor(
                        out=ou_all[0:w, tt * d:tt * d + d], in0=trp[0:w, 0:d],
                        scalar=gam[0:w, tt:tt + 1],
                        in1=o1_all[0:w, tt * d:tt * d + d], op0=mult, op1=add,
                    )
                nc.sync.dma_start(
                    out=out[0:(nt3 - 1) * P, :]
                        .rearrange("(t p) x -> p t x", p=P),
                    in_=ou_all[:, 0:(nt3 - 1) * d]
                        .rearrange("p (t x) -> p t x", x=d),
                )
                wlast = cs - (nt3 - 1) * P
                nc.scalar.dma_start(
                    out=out[(nt3 - 1) * P:cs, :],
                    in_=ou_all[0:wlast, (nt3 - 1) * d:nt3 * d],
                )

    nc.compile()
    return nc


_PROGRAM = None
LAST_RESULTS = None


def _get_program():
    global _PROGRAM
    if _PROGRAM is None:
        _PROGRAM = build_program()
    return _PROGRAM


def kernel(embeded_preds, labels, center):
    from concourse.bass_utils import run_bass_kernel_spmd

    global LAST_RESULTS
    preds = np.ascontiguousarray(np.asarray(embeded_preds, dtype=np.float32))
    lab = np.ascontiguousarray(np.asarray(labels, dtype=np.float32))
    ctr = np.ascontiguousarray(np.asarray(center, dtype=np.float32))
    assert preds.shape == (N, D) and lab.shape == (N, C) and ctr.shape == (C, D)

    nc = _get_program()
    preds_nat = preds.reshape(P, KT * D)

    def permute_center(cj):
        cpad = np.zeros((NT3 * P, D), dtype=np.float32)
        cpad[:cj.shape[0]] = cj
        return np.ascontiguousarray(
            cpad.reshape(NT3, P, D).transpose(1, 0, 2).reshape(P, NT3 * D)
        )

    in_maps = [
        {
            "preds": preds_nat,
            "labels": np.ascontiguousarray(lab[:, j * CS:(j + 1) * CS])
                .reshape(N // 2, 2 * CS),
            "center": permute_center(ctr[j * CS:(j + 1) * CS]),
        }
        for j in range(NCORES)
    ]
    res = run_bass_kernel_spmd(nc, in_maps, core_ids=list(range(NCORES)))
    LAST_RESULTS = res
    return np.concatenate([res.results[j]["out"] for j in range(NCORES)], axis=0)


# revision 23
# speedup vs baseline: 1.0317x; 1.0317x over previous
"""CenterLoss update kernel for Trainium2, 8-core SPMD — class-sharded (baseline control)."""

import numpy as np

N, C, D = 16384, 10000, 128
NCORES = 8
CS = C // NCORES   # 1250 classes per core
LR = 0.5
P = 128
KT = N // P        # 128 virtual k-tiles
NPAIR = KT // 2    # 64 pair-tiles
NT3 = (CS + P - 1) // P  # output tiles over the class shard
PCHUNKS = [1024] * 16   # preds load chunks (cols of the [128, N] natural view)
QQSPLIT = 56            # counts accumulator split point (pair index)
assert sum(PCHUNKS) == KT * D


def _chunks(width, step=512):
    out = []
    c0 = 0
    while c0 < width:
        out.append((c0, min(step, width - c0)))
        c0 += step
    return out


def build_program(cs=CS, d=D, kt=KT):
    """Build the SPMD Bass program (identical on every core)."""
    import concourse.bacc as bacc
    import concourse.mybir as mybir
    import concourse.tile as tile
    from concourse.masks import make_identity

    f32 = mybir.dt.float32
    f32r = mybir.dt.float32r
    mult = mybir.AluOpType.mult
    add = mybir.AluOpType.add

    n = kt * P
    nt3 = NT3
    npair = NPAIR
    assert cs * 4 <= 3 * 2048, "S.T PSUM tile must fit in 3 banks"

    nc = bacc.Bacc(
        "TRN2",
        target_bir_lowering=False,
        debug=False,
        num_devices=NCORES,
    )

    preds = nc.dram_tensor("preds", [P, kt * d], f32r, kind="ExternalInput").ap()
    labels2 = nc.dram_tensor(
        "labels", [n // 2, 2 * cs], f32r, kind="ExternalInput"
    ).ap()
    center = nc.dram_tensor("center", [P, nt3 * d], f32, kind="ExternalInput").ap()
    out = nc.dram_tensor("out", [cs, d], f32, kind="ExternalOutput").ap()

    trigger_qq = {}
    for cch in range(len(PCHUNKS)):
        trigger_qq.setdefault(max(0, 4 * cch - 2), []).append(cch)

    with tile.TileContext(nc) as tc:
        with tc.tile_pool(name="const", bufs=1) as const_pool:
            identity = const_pool.tile([P, P], f32, name="identity")
            make_identity(nc, identity[:])
            ones_col = const_pool.tile([P, 1], f32, name="ones_col")
            nc.vector.memset(ones_col[:], 1.0)

            ctr_sb = const_pool.tile([P, nt3 * d], f32, name="ctr_sb")
            nc.gpsimd.dma_start(out=ctr_sb[:], in_=center[:])

            preds_hi = [
                const_pool.tile([P, pw], f32r, name=f"preds_hi_{cch}")
                for cch, pw in enumerate(PCHUNKS)
            ]
            pstart = [sum(PCHUNKS[:cch]) for cch in range(len(PCHUNKS))]

            acc_a = const_pool.tile([P, 2 * cs], f32, name="acc_a")
            acc_c = const_pool.tile([P, 2 * cs], f32, name="acc_c")

            st_sb = const_pool.tile([d, cs], f32, name="st_sb")
            cnt_row = const_pool.tile([1, cs], f32, name="cnt_row")

            with (
                tc.tile_pool(name="lab", bufs=7) as lab_pool,
                tc.tile_pool(name="psum1", bufs=1, space="PSUM") as psum1,
            ):
                st_psum = psum1.tile([d, cs], f32, name="st_psum", space="PSUM")
                cnt_psum = psum1.tile([1, cs], f32, name="cnt_psum", space="PSUM")
                for qq in range(npair):
                    for cch in trigger_qq.get(qq, []):
                        peng = nc.sync if cch % 2 == 0 else nc.scalar
                        peng.dma_start(
                            out=preds_hi[cch][:],
                            in_=preds[:, pstart[cch]:pstart[cch] + PCHUNKS[cch]],
                        )
                    lab2 = lab_pool.tile(
                        [P, 2 * cs], f32r, name=f"lab_{qq}", tag="lab"
                    )
                    eng = nc.sync if qq % 2 == 0 else nc.scalar
                    eng.dma_start(out=lab2[:], in_=labels2[qq::npair, :])
                    for h in (0, 1):
                        q = 2 * qq + h
                        col = q * d
                        cch = max(
                            i for i in range(len(PCHUNKS)) if pstart[i] <= col
                        )
                        for c0, w in _chunks(cs):
                            nc.tensor.matmul(
                                out=st_psum[:, c0:c0 + w],
                                lhsT=preds_hi[cch][:, col - pstart[cch]:
                                                   col - pstart[cch] + d],
                                rhs=lab2[:, h * cs + c0:h * cs + c0 + w],
                                start=(q == 0),
                                stop=(q == kt - 1),
                            )
                    acc = acc_a if qq < QQSPLIT else acc_c
                    if qq in (0, QQSPLIT):
                        nc.vector.tensor_copy(out=acc[:], in_=lab2[:].bitcast(f32))
                    else:
                        nc.vector.tensor_add(
                            out=acc[:], in0=acc[:], in1=lab2[:].bitcast(f32)
                        )
                    if qq == QQSPLIT + 2:
                        for h in (0, 1):
                            for c0, w in _chunks(cs):
                                nc.tensor.matmul(
                                    out=cnt_psum[0:1, c0:c0 + w],
                                    lhsT=ones_col[:],
                                    rhs=acc_a[:, h * cs + c0:h * cs + c0 + w],
                                    start=(h == 0),
                                    stop=False,
                                )

                for h in (0, 1):
                    for c0, w in _chunks(cs):
                        nc.tensor.matmul(
                            out=cnt_psum[0:1, c0:c0 + w],
                            lhsT=ones_col[:],
                            rhs=acc_c[:, h * cs + c0:h * cs + c0 + w],
                            start=False,
                            stop=(h == 1),
                        )
                nc.scalar.copy(out=cnt_row[:], in_=cnt_psum[:])
                nc.scalar.copy(out=st_sb[:], in_=st_psum[:])

            with (
                tc.tile_pool(name="p3", bufs=2) as p3,
                tc.tile_pool(name="psum3", bufs=1, space="PSUM") as psum3,
            ):
                cnt_all = psum3.tile([P, nt3], f32, name="cnt_all", space="PSUM")
                for tt in range(nt3):
                    w = min(P, cs - tt * P)
                    nc.tensor.transpose(
                        out=cnt_all[0:w, tt:tt + 1],
                        in_=cnt_row[0:1, tt * P:tt * P + w],
                        identity=identity[0:1, 0:1],
                    )
                den = p3.tile([P, nt3], f32, name="den", tag="den", bufs=1)
                nc.vector.tensor_scalar_add(out=den[:], in0=cnt_all[:], scalar1=1.0)
                rec = p3.tile([P, nt3], f32, name="rec", tag="rec", bufs=1)
                nc.vector.reciprocal(out=rec[:], in_=den[:])
                gam = p3.tile([P, nt3], f32, name="gam", tag="gam", bufs=1)
                nc.vector.tensor_scalar_mul(out=gam[:], in0=rec[:], scalar1=0.5)
                bet = p3.tile([P, nt3], f32, name="bet", tag="bet", bufs=1)
                nc.vector.tensor_tensor(
                    out=bet[:], in0=cnt_all[:], in1=rec[:], op=mult
                )
                nc.vector.tensor_scalar(
                    out=bet[:], in0=bet[:],
                    scalar1=-0.5, scalar2=1.0, op0=mult, op1=add,
                )

                o1_all = p3.tile([P, nt3 * d], f32, name="o1_all", tag="o1",
                                 bufs=1)
                nc.vector.tensor_tensor(
                    out=o1_all[:].rearrange("p (t x) -> p t x", x=d),
                    in0=ctr_sb[:].rearrange("p (t x) -> p t x", x=d),
                    in1=bet[:].unsqueeze(2).broadcast_to([P, nt3, d]),
                    op=mult,
                )

                ou_all = p3.tile([P, nt3 * d], f32, name="ou_all", tag="ou",
                                 bufs=1)
                for tt in range(nt3):
                    w = min(P, cs - tt * P)
                    trp = psum3.tile([P, d], f32, name=f"trp_{tt}", tag="trp",
                                     bufs=4, space="PSUM")
                    nc.tensor.transpose(
                        out=trp[0:w, 0:d],
                        in_=st_sb[:, tt * P:tt * P + w],
                        identity=identity[:, 0:d],
                    )
                    nc.vector.scalar_tensor_tensor(
                        out=ou_all[0:w, tt * d:tt * d + d], in0=trp[0:w, 0:d],
                        scalar=gam[0:w, tt:tt + 1],
                        in1=o1_all[0:w, tt * d:tt * d + d], op0=mult, op1=add,
                    )
                nc.sync.dma_start(
                    out=out[0:(nt3 - 1) * P, :]
                        .rearrange("(t p) x -> p t x", p=P),
                    in_=ou_all[:, 0:(nt3 - 1) * d]
                        .rearrange("p (t x) -> p t x", x=d),
                )
                wlast = cs - (nt3 - 1) * P
                nc.scalar.dma_start(
                    out=out[(nt3 - 1) * P:cs, :],
                    in_=ou_all[0:wlast, (nt3 - 1) * d:nt3 * d],
                )

    nc.compile()
    return nc


_PROGRAM = None
LAST_RESULTS = None


def _get_program():
    global _PROGRAM
    if _PROGRAM is None:
        _PROGRAM = build_program()
    return _PROGRAM


def kernel(embeded_preds, labels, center):
    from concourse.bass_utils import run_bass_kernel_spmd

    global LAST_RESULTS
    preds = np.ascontiguousarray(np.asarray(embeded_preds, dtype=np.float32))
    lab = np.ascontiguousarray(np.asarray(labels, dtype=np.float32))
    ctr = np.ascontiguousarray(np.asarray(center, dtype=np.float32))
    assert preds.shape == (N, D) and lab.shape == (N, C) and ctr.shape == (C, D)

    nc = _get_program()
    preds_nat = preds.reshape(P, KT * D)

    def permute_center(cj):
        cpad = np.zeros((NT3 * P, D), dtype=np.float32)
        cpad[:cj.shape[0]] = cj
        return np.ascontiguousarray(
            cpad.reshape(NT3, P, D).transpose(1, 0, 2).reshape(P, NT3 * D)
        )

    in_maps = [
        {
            "preds": preds_nat,
            "labels": np.ascontiguousarray(lab[:, j * CS:(j + 1) * CS])
                .reshape(N // 2, 2 * CS),
            "center": permute_center(ctr[j * CS:(j + 1) * CS]),
        }
        for j in range(NCORES)
    ]
    res = run_bass_kernel_spmd(nc, in_maps, core_ids=list(range(NCORES)))
    LAST_RESULTS = res
    return np.concatenate([res.results[j]["out"] for j in range(NCORES)], axis=0)
